# revision 15
# baseline (speedup 1.0000x reference)
"""Trainium2 Bass kernel for unscaled dot-product attention.

Shapes (hardcoded): query/key/value [2048, 2, 16, 64] fp32.
  scores = einsum('sbnh,tbnh->bnst', q, k)   (UNscaled)
  probs  = softmax(scores, axis=-1)
  out    = einsum('bnst,tbnh->sbnh', probs, v).reshape(2048, 2, 1024)

Sharding: the 32 (b, n) head-slices are split 4-per-core across 8 cores
(core c -> b = c//4, heads 4*(c%4) .. +4). Each core computes attention
for its 4 heads independently; no cross-device communication.

Device-side strategy (per core, heads processed in 2 pairs):
  - Q^T/K^T are [head*64+h, s] fp32r; V' is [t, 66] per t-block per head
    in bf16 with a ones column (the PV matmul then produces the softmax
    denominator for free) and a zero pad column.
  - Inputs are staged as per-section DRAM tensors and DMA-ed in use
    order (K/Q chunk 0 first) so the first QK matmul starts ~1.5us in.
  - scores are computed TRANSPOSED: scoresT[t_block, s] = K^T.T @ Q^T
    per 128-t block; the two heads of a pair use disjoint 64-row halves
    of the PE array (row tile_position) writing separate 1-bank PSUM
    score tiles.
  - Work is scheduled at HALF-step granularity (one head x one 512-col
    s-chunk x one t-block). Per half, exp() runs on one of two engines
    concurrently: the Scalar (ACT) engine computes true exp() from
    PSUM; the Vector (DVE) engine computes a one-instruction
    Schraudolph approximation (int16(x*128*log2e + magic) bit-cast to
    bf16). Softmax normalizes away most of the approximation error:
    end-to-end rel-l2 stays ~4.5e-3 with ~half the blocks on DVE.
    No max-subtraction: unscaled scores peak ~+-66, safely in range.
  - PV accumulates CT[66, s] = V'.T @ expT over the 16 t-blocks in PSUM
    (rows 0-63 context^T, row 64 = denominator, row 65 = padding).
  - CT (unnormalized) + denominator row are evacuated (head A on ACT,
    head B on DVE, in parallel) and DMA-ed to HBM; the final divide +
    transpose happens on the host.
"""

import numpy as np

SQ, B, NHEADS, HN = 2048, 2, 16, 64
N_CORES = 8
HEADS_PER_CORE = 4
VW = 66                     # V' columns per head (64 V + ones + pad)

LOG2E = 1.4426950408889634
EXP_A16 = 128.0 * LOG2E
EXP_B16 = 128.0 * 127.0 - 7.42

_CACHE = {}


def _round_fp32r(x):
    """Round fp32 array to the fp32r grid (11 explicit mantissa bits,
    round-to-nearest-even, low 12 bits zero)."""
    u = np.ascontiguousarray(x, np.float32).view(np.uint32)
    lsb = (u >> 12) & 1
    u = (u + 0x7FF + lsb) & 0xFFFFF000
    return u.astype(np.uint32).view(np.float32)


def dve_half(j, p):
    """Engine assignment for exp of half (t-block j, head-slot p)."""
    return (j + p) % 2 == 1 and (j, p) != (0, 1)


def _build_program(reps=1, lookahead=1, sc_bufs=3, qk_dt="bfloat16",
                   exp_mode="split", vw=128, evac_mode="full",
                   pool_mode="big", pv_mode="full", n_dve=7,
                   dma_hoist=False):
    from contextlib import ExitStack

    import concourse.bacc as bacc
    import concourse.mybir as mybir
    import concourse.tile as tile

    f32 = mybir.dt.float32
    f32r = mybir.dt.float32r
    bf16 = mybir.dt.bfloat16
    i16 = mybir.dt.int16
    EXP = mybir.ActivationFunctionType.Exp
    COPY = mybir.ActivationFunctionType.Copy
    MUL = mybir.AluOpType.mult
    ADD = mybir.AluOpType.add

    nc = bacc.Bacc("TRN2", target_bir_lowering=False, debug=False,
                   num_devices=N_CORES)

    qkd = getattr(mybir.dt, qk_dt)
    ins_q = nc.dram_tensor("ins_q", [2, 4, 128, 512], qkd,
                           kind="ExternalInput").ap()
    ins_k = nc.dram_tensor("ins_k", [2, 4, 128, 512], qkd,
                           kind="ExternalInput").ap()
    ins_v = nc.dram_tensor("ins_v", [2, 2, 128, 8 * 2 * vw], bf16,
                           kind="ExternalInput").ap()
    # outT rows: head h in 0..3 occupies [65h, 65h+65): 64 context^T rows
    # + 1 denominator row. Host divides and transposes.
    outT = nc.dram_tensor("outT", [4 * 65, SQ], f32, kind="ExternalOutput").ap()

    SCH = 512            # s-chunk processed per inner loop
    NCH = SQ // SCH      # 4 chunks
    NT = SQ // 128       # 16 t-blocks

    with tile.TileContext(nc) as tc, ExitStack() as ctx:
        in_pool = ctx.enter_context(tc.tile_pool(name="inp", bufs=2))
        ex_pool = ctx.enter_context(tc.tile_pool(name="ex", bufs=4))
        fin_pool = ctx.enter_context(tc.tile_pool(name="fin", bufs=2))
        # PSUM budget (8 banks of [128 x 512 fp32]):
        #   scores 2 tags x sc_bufs x 1 bank + CT 2 tags x 1 bank
        if pool_mode == "big":
            ps_sc = ctx.enter_context(
                tc.tile_pool(name="ps_sc", bufs=3, space="PSUM"))
            ps_ct = ctx.enter_context(tc.tile_pool(name="ps_ct", bufs=1,
                                                   space="PSUM"))
        elif pool_mode == "rot":
            ps_sc = ctx.enter_context(
                tc.tile_pool(name="ps_sc", bufs=5, space="PSUM"))
            ps_ct = ctx.enter_context(tc.tile_pool(name="ps_ct", bufs=3,
                                                   space="PSUM"))
        else:
            ps_sc = ctx.enter_context(
                tc.tile_pool(name="ps_sc", bufs=sc_bufs, space="PSUM"))
            ps_ct = ctx.enter_context(tc.tile_pool(name="ps_ct", bufs=1,
                                                   space="PSUM"))

        rep_cm = tc.For_i(0, reps, 1) if reps != 1 else None

        def load_pairs():
            # Per pair: K/Q/V split in 512-col (K, Q) / 1056-col (V)
            # pieces, issued in first-use order so compute starts early.
            pair = []
            for g in range(2):
                kt = [in_pool.tile([128, 512], qkd, tag=f"kt{i}",
                                   name=f"kt{i}") for i in range(4)]
                qt = [in_pool.tile([128, 512], qkd, tag=f"qt{i}",
                                   name=f"qt{i}") for i in range(4)]
                vt = [in_pool.tile([128, 8 * 2 * vw], bf16, tag=f"vt{i}",
                                   name=f"vt{i}") for i in range(2)]
                order = [("k", 0), ("q", 0), ("v", 0), ("k", 1), ("v", 1),
                         ("k", 2), ("k", 3), ("q", 1), ("q", 2), ("q", 3)]
                for sec, i in order:
                    if sec == "k":
                        nc.sync.dma_start(out=kt[i][:], in_=ins_k[g][i])
                    elif sec == "q":
                        nc.sync.dma_start(out=qt[i][:], in_=ins_q[g][i])
                    else:
                        nc.sync.dma_start(out=vt[i][:], in_=ins_v[g][i])
                v3 = [v.rearrange("p (j c) -> p j c", c=2 * vw) for v in vt]
                pair.append((qt, kt, v3))
            return pair

        if dma_hoist:
            pair = load_pairs()
        if rep_cm is not None:
            ctx.enter_context(rep_cm)
        if not dma_hoist:
            pair = load_pairs()

        steps = [(g, c, j) for g in range(2) for c in range(NCH)
                 for j in range(NT)]

        def emit_qk(s):
            """Emit both QK halves of step s; returns (sc_a, sc_b)."""
            g, c, j = steps[s]
            qt, kt, _ = pair[g]
            kb = kt[j // 4][:, (j % 4) * 128:(j % 4) * 128 + 128]
            if pool_mode == "big":
                scw = ps_sc.tile([128, 1024], f32, tag="sc", name="scw")
                sca = scw[:, 0:512]
                scb = scw[:, 512:1024]
            elif pool_mode == "rot":
                sca = ps_sc.tile([128, 512], f32, tag="sc", name="sca")
                scb = ps_sc.tile([128, 512], f32, tag="sc", name="scb")
            else:
                sca = ps_sc.tile([128, 512], f32, tag="sca", name="sca")
                scb = ps_sc.tile([128, 512], f32, tag="scb", name="scb")
            nc.tensor.matmul(sca[:], lhsT=kb[0:64, :], rhs=qt[c][0:64, :],
                             start=True, stop=True)
            nc.tensor.matmul(scb[:], lhsT=kb[64:128, :], rhs=qt[c][64:128, :],
                             start=True, stop=True)
            return sca, scb

        const_ex = [None]
        DVE_JS = set([1, 3, 5, 7, 9, 11, 13, 15, 0, 2, 4, 6][:n_dve])

        def emit_exp_big(s, scs):
            g, c, j = steps[s]
            ex = ex_pool.tile([128, 1024], bf16, tag="exw", name="exw")
            sca = scs[0]
            if j in DVE_JS:
                nc.vector.tensor_scalar(
                    out=ex.bitcast(i16)[:], in0=sca.tensor.ap()[:],
                    scalar1=EXP_A16, scalar2=EXP_B16, op0=MUL, op1=ADD)
            else:
                nc.scalar.activation(ex[:], sca.tensor.ap()[:], EXP)
            return [ex[:, 0:512], ex[:, 512:1024]]

        def emit_exp(s, scs):
            g, c, j = steps[s]
            if pool_mode == "big" and exp_mode == "split":
                return emit_exp_big(s, scs)
            if exp_mode == "skip":
                if const_ex[0] is None:
                    cx = ex_pool.tile([128, 512], bf16, tag="cex",
                                      name="cex")
                    nc.gpsimd.memset(cx[:], 0.25)
                    const_ex[0] = cx
                return [const_ex[0], const_ex[0]]
            exs = []
            for p in (0, 1):
                ex = ex_pool.tile([128, 512], bf16, tag=f"ex{p}",
                                  name=f"ex{p}")
                use_dve = (dve_half(j, p) if exp_mode == "split"
                           else exp_mode == "dve")
                if use_dve:
                    nc.vector.tensor_scalar(
                        out=ex.bitcast(i16)[:], in0=scs[p][:],
                        scalar1=EXP_A16, scalar2=EXP_B16, op0=MUL, op1=ADD)
                else:
                    nc.scalar.activation(ex[:], scs[p][:], EXP)
                exs.append(ex)
            return exs

        CT = [None, None]

        def emit_pv(s, exs):
            g, c, j = steps[s]
            _, _, v3 = pair[g]
            vb = v3[j // 8]
            for p in (0, 1):
                if j == 0:
                    ctag = "ct" if pool_mode == "rot" else f"ct{p}"
                    CT[p] = ps_ct.tile([vw, 512], f32, tag=ctag,
                                       name=f"ct{p}")
            if pv_mode == "rowsplit":
                # Interleave heads x K-halves so adjacent matmuls are
                # row-disjoint (A1||B1, B2||A2 can overlap in the PE array).
                wA = vb[:, j % 8, 0:vw]
                wB = vb[:, j % 8, vw:2 * vw]
                st, sp = (j == 0), (j == NT - 1)
                nc.tensor.matmul(CT[0][:], lhsT=wA[0:64, :],
                                 rhs=exs[0][0:64, :], start=st, stop=False)
                nc.tensor.matmul(CT[1][:], lhsT=wB[64:128, :],
                                 rhs=exs[1][64:128, :], start=st, stop=False)
                nc.tensor.matmul(CT[1][:], lhsT=wB[0:64, :],
                                 rhs=exs[1][0:64, :], start=False, stop=sp)
                nc.tensor.matmul(CT[0][:], lhsT=wA[64:128, :],
                                 rhs=exs[0][64:128, :], start=False, stop=sp)
                return
            for p in (0, 1):
                nc.tensor.matmul(
                    CT[p][:],
                    lhsT=vb[:, j % 8, p * vw:(p + 1) * vw],
                    rhs=exs[p][:],
                    start=(j == 0), stop=(j == NT - 1))

        def emit_evac(s):
            """After step (g, c, 15): evacuate both CT tiles + DMA out.
            Head A on ACT, head B on DVE, in parallel."""
            g, c, j = steps[s]
            s0 = c * SCH
            if evac_mode == "skip":
                return
            for p, CTs in enumerate(CT):
                cte = fin_pool.tile([65, 512], f32, tag=f"cte{p}",
                                    name=f"cte{p}")
                if p == 0:
                    nc.scalar.activation(cte[:], CTs[0:65, :], COPY)
                else:
                    nc.vector.tensor_copy(cte[:], CTs[0:65, :])
                h = 2 * g + p
                nc.sync.dma_start(out=outT[65 * h:65 * h + 65, s0:s0 + SCH],
                                  in_=cte[:])

        # Prologue: prime the QK pipeline `lookahead` steps ahead.
        scq = [emit_qk(s) for s in range(lookahead)]
        for s in range(len(steps)):
            if s + lookahead < len(steps):
                scq.append(emit_qk(s + lookahead))
            exs = emit_exp(s, scq[s])
            emit_pv(s, exs)
            if steps[s][2] == NT - 1:
                emit_evac(s)
    nc.compile()
    return nc


def get_nc(reps=1, lookahead=1, sc_bufs=3, qk_dt="bfloat16",
           exp_mode="split", vw=128, evac_mode="full", pool_mode="big",
           pv_mode="full", n_dve=7, dma_hoist=False):
    key = ("nc", reps, lookahead, sc_bufs, qk_dt, exp_mode, vw, evac_mode,
           pool_mode, pv_mode, n_dve, dma_hoist)
    if key not in _CACHE:
        _CACHE[key] = _build_program(reps, lookahead, sc_bufs, qk_dt,
                                     exp_mode, vw, evac_mode, pool_mode,
                                     pv_mode, n_dve, dma_hoist)
    return _CACHE[key]


def make_in_maps(query, key, value, qk_dt="bfloat16", vw=128):
    """Host-side sharding + layout prep. Returns list of per-core input maps."""
    import ml_dtypes
    query = np.asarray(query, dtype=np.float32)
    key = np.asarray(key, dtype=np.float32)
    value = np.asarray(value, dtype=np.float32)
    in_maps = []
    for c in range(N_CORES):
        b = c // 4
        n0 = HEADS_PER_CORE * (c % 4)
        q = query[:, b, n0:n0 + 4, :]   # [2048, 4, 64]
        k = key[:, b, n0:n0 + 4, :]
        v = value[:, b, n0:n0 + 4, :]
        if qk_dt == "float32r":
            qt = _round_fp32r(q.transpose(1, 2, 0).reshape(2, 128, SQ))
            kt = _round_fp32r(k.transpose(1, 2, 0).reshape(2, 128, SQ))
        else:
            qt = np.ascontiguousarray(
                q.transpose(1, 2, 0).reshape(2, 128, SQ)).astype(
                    ml_dtypes.bfloat16)
            kt = np.ascontiguousarray(
                k.transpose(1, 2, 0).reshape(2, 128, SQ)).astype(
                    ml_dtypes.bfloat16)
        # piece-major so each device DMA is fully contiguous
        qt = np.ascontiguousarray(
            qt.reshape(2, 128, 4, 512).transpose(0, 2, 1, 3))
        kt = np.ascontiguousarray(
            kt.reshape(2, 128, 4, 512).transpose(0, 2, 1, 3))
        # V' [2048, 4, 66] -> [16, 128, 2 pairs, 132] -> [2, 128, 16*132]
        vp = np.concatenate(
            [v, np.ones((SQ, 4, 1), np.float32),
             np.zeros((SQ, 4, vw - 65), np.float32)], axis=2)
        vp = vp.reshape(16, 128, 2, 2 * vw).transpose(2, 0, 1, 3)
        vp = vp.reshape(2, 2, 8, 128, 2 * vw).transpose(0, 1, 3, 2, 4)
        vp = np.ascontiguousarray(vp.reshape(2, 2, 128, 8 * 2 * vw)).astype(
            ml_dtypes.bfloat16)
        in_maps.append({"ins_q": np.ascontiguousarray(qt),
                        "ins_k": np.ascontiguousarray(kt),
                        "ins_v": vp})
    return in_maps


def assemble_output(results):
    """results: list of per-core {name: array} dicts -> full [2048, 2, 1024].

    Per-core outT is [4*65, 2048]: per head 64 unnormalized context^T rows
    + 1 denominator row; divide + transpose here.
    """
    out = np.empty((SQ, B, NHEADS, HN), np.float32)
    for c in range(N_CORES):
        b = c // 4
        n0 = HEADS_PER_CORE * (c % 4)
        oT = np.asarray(results[c]["outT"]).reshape(4, 65, SQ)
        ctx = oT[:, 0:64, :] / oT[:, 64:65, :]
        out[:, b, n0:n0 + 4, :] = ctx.transpose(2, 0, 1)
    return out.reshape(SQ, B, NHEADS * HN)


def kernel(query, key, value):
    try:
        from concourse.bass_utils import run_bass_kernel_spmd
    except ImportError:
        import sys
        sys.path.insert(0, "/opt/trn_rl_repo")
        from concourse.bass_utils import run_bass_kernel_spmd

    nc = get_nc()
    in_maps = make_in_maps(query, key, value)
    res = run_bass_kernel_spmd(nc, in_maps, list(range(N_CORES)))
    return assemble_output(res.results)
